# revision 3
# baseline (speedup 1.0000x reference)
"""CapsNet forward kernel for Trainium2, 8-core data-parallel (v2, bf16).

Strategy: batch (512) split across 8 cores (64 each); params replicated.
Routing logits b are a batch-mean -> AllGather of per-core partial deltas
(1152 floats) per routing round (rounds 0,1; round 2's b update is dead).

v2 changes vs baseline:
  - all PE matmuls in bf16 (same streaming rate as fp32r at N>=256, 4x
    faster at the routing's small-N matmuls; half the DMA bytes).
  - conv runs in two 32-image halves so h1 (both ci blocks, bf16) fits
    in SBUF; conv2 accumulates K=256 in a single PSUM chain (no acc add).
  - conv1 bias+relu alternates Act/DVE engines; conv2 group 0 is emitted
    between conv1 chunks so the PE starts conv2 as soon as its inputs
    land instead of idling on patch DMAs.
  - patch rows pre-expanded on host (xp[81,b,560]) so patch DMAs are
    3-dim and batched; half-1 patches prefetched during half 0.
  - routing weights (w2nt) DMA'd into the conv weight pool's freed slots
    during the last conv section; xrT round-trip split in two tiles and
    overlapped with the last conv stores.
  - P*xr reduction grouped 8 r-tiles per PSUM bank, consumed by two DVE
    ops per group (no per-tile PSUM->SBUF copies).
  - per-round scaled bf16 copies of xrT (master stays unscaled -> no
    reciprocal un-scaling).
"""

import numpy as np

import concourse.bass as bass
import concourse.mybir as mybir
import concourse.tile as tile
from concourse.ap import AP
from concourse.bass_utils import run_bass_kernel_spmd

F32 = mybir.dt.float32
BF16 = mybir.dt.bfloat16
AL = mybir.AluOpType
AF = mybir.ActivationFunctionType
AX = mybir.AxisListType

NCORES = 8
B = 512
BC = B // NCORES           # 64 images per core
HB = BC // 2               # 32 images per conv half
MAX_WAITS = 1              # walrus on this path allows 1 sync wait per inst
HL = 160                   # 10 classes x 16 pose
NS = 9216                  # 1152 caps x 8
NT = NS // 128             # 72 K-tiles
HT = NT // 2               # 36 K-tiles per xrT half
GROUPS = [(0, 14), (14, 14), (28, 4)]   # conv2 image groups per half
PATCH_CHUNKS = [(0, 7), (7, 7), (14, 7), (21, 7), (28, 4)]  # conv1 DMA chunks
ROUTE_SCALE = 1.0 / (B * HL)


def _r(t, dims):
    """Raw AP on tile/ap t with explicit [step, count] dims (elements)."""
    return AP(t.tensor, t.offset, dims)


def split_waits(nc, max_waits=MAX_WAITS):
    """This walrus build rejects >max_waits sync waits per instruction; move
    excess waits onto same-engine NoOps inserted immediately before."""
    for f in nc.m.functions:
        for blk in f.blocks:
            out = []
            for ins in blk.instructions:
                si = ins.sync_info
                if si is not None and si.on_wait and len(si.on_wait) > max_waits:
                    waits = list(si.on_wait)
                    k = 0
                    while len(waits) > max_waits:
                        chunk, waits = waits[:max_waits], waits[max_waits:]
                        nop = mybir.InstNoOp(name=f"{ins.name}-ws{k}", ins=[], outs=[])
                        nop.engine = ins.engine
                        nop.sync_info = mybir.SyncInfo(on_wait=chunk, on_update=[])
                        out.append(nop)
                        k += 1
                    ins.sync_info = mybir.SyncInfo(
                        on_wait=waits, on_update=list(si.on_update or []))
                out.append(ins)
            blk.instructions = out


def build_nc():
    nc = bass.Bass(num_devices=NCORES)

    xp = nc.dram_tensor("xp", [81, BC, 560], BF16, kind="ExternalInput")
    w1t = nc.dram_tensor("w1t", [81, 256], BF16, kind="ExternalInput")
    b1 = nc.dram_tensor("b1", [256], F32, kind="ExternalInput")
    pcw4 = nc.dram_tensor("pcw4", [4, 128, 81 * 128], BF16, kind="ExternalInput")
    pcb = nc.dram_tensor("pcb", [256], F32, kind="ExternalInput")
    w2ns = nc.dram_tensor("w2ns", [128, NT * HL], BF16, kind="ExternalInput")
    w2nt = nc.dram_tensor("w2nt", [HL, NS], BF16, kind="ExternalInput")
    eye64 = nc.dram_tensor("eye64", [BC, BC], BF16, kind="ExternalInput")
    vout = nc.dram_tensor("vout", [BC, HL], F32, kind="ExternalOutput")

    pc_rd = nc.dram_tensor("pc_rd", [NS, BC], BF16)    # [r, b]

    with tile.TileContext(nc) as tc:
        with (
            tc.tile_pool(name="pers", bufs=1) as pers,
            tc.tile_pool(name="dram", bufs=1, space="DRAM") as dpool,
        ):
            w1t_sb = pers.tile([81, 256], BF16)
            nc.sync.dma_start(w1t_sb[:], w1t[:])
            b1_sb = pers.tile([128, 2], F32)
            nc.sync.dma_start(b1_sb[:], _r(b1[:], [[1, 128], [128, 2]]))
            pcb_sb = pers.tile([128, 2], F32)
            nc.sync.dma_start(pcb_sb[:], _r(pcb[:], [[1, 128], [128, 2]]))
            eye_sb = pers.tile([BC, BC], BF16)
            zero1 = pers.tile([128, 1], F32)
            nc.gpsimd.memset(zero1[:], 0.0)
            ones128 = pers.tile([128, 1], F32)
            nc.gpsimd.memset(ones128[:], 1.0)
            ones1 = pers.tile([1, 128], F32)
            nc.gpsimd.memset(ones1[:], 1.0)
            b9 = pers.tile([128, 9], F32)
            # big persistent routing tensors (DMAs issued later, mid-conv)
            w2sb = pers.tile([128, NT * HL], BF16)
            xrT_a = pers.tile([128, HT * BC], BF16)
            xrT_b = pers.tile([128, HT * BC], BF16)

            with tc.tile_pool(name="wbig", bufs=1) as wbig:
                # ---------------- conv phase ----------------
                with (
                    tc.tile_pool(name="convsb", bufs=1) as csb,
                    tc.tile_pool(name="pwp", bufs=2) as pwp,
                    tc.tile_pool(name="pc2p", bufs=2) as pc2p,
                    tc.tile_pool(name="ps1p", bufs=4, space="PSUM") as ps1p,
                    tc.tile_pool(name="ps2p", bufs=2, space="PSUM") as ps2p,
                ):
                    h1_0 = csb.tile([128, HB * 400], BF16)
                    h1_1 = csb.tile([128, HB * 400], BF16)
                    h1 = [h1_0, h1_1]
                    w2c = {}
                    for (co, ci) in [(0, 0), (0, 1), (1, 0), (1, 1)]:
                        t = wbig.tile([128, 81 * 128], BF16, tag="wslot",
                                      bufs=4, name=f"w2c_{co}{ci}")
                        w2c[(co, ci)] = t

                    def load_w2c(co, ci):
                        nc.sync.dma_start(
                            w2c[(co, ci)][:],
                            AP(pcw4[:].tensor, (co * 2 + ci) * 128 * 81 * 128,
                               [[81 * 128, 128], [1, 81 * 128]]),
                        )

                    def patch_dma(half, lo, n):
                        """DMA patch rows for images half*32+lo .. +n."""
                        pw = pwp.tile([81, 8 * 560], BF16, tag="pw")
                        nc.sync.dma_start(
                            _r(pw, [[pw.ap[0][0], 81], [1, n * 560]]),
                            AP(xp[:].tensor, (half * HB + lo) * 560,
                               [[BC * 560, 81], [560, n], [1, 560]]),
                        )
                        return pw

                    def conv1_imgs(pw, lo, n):
                        """conv1 matmuls/acts for the n images in patch pw."""
                        pwstep = pw.ap[0][0]
                        for j in range(n):
                            for ci in range(2):
                                ps1 = ps1p.tile([128, 400], F32, tag="ps1")
                                rhs = AP(pw.tensor, pw.offset + j * 560,
                                         [[pwstep, 81], [28, 20], [1, 20]])
                                out4 = _r(ps1, [[ps1.ap[0][0], 128],
                                                [20, 20], [1, 20]])
                                nc.tensor.matmul(
                                    out4,
                                    w1t_sb[:, ci * 128:(ci + 1) * 128],
                                    rhs,
                                    start=True, stop=True,
                                )
                                il = lo + j
                                dst = h1[ci][:, il * 400:(il + 1) * 400]
                                if ci == 0:
                                    nc.scalar.activation(
                                        dst, ps1[:], AF.Relu,
                                        bias=b1_sb[:, 0:1],
                                    )
                                else:
                                    nc.vector.scalar_tensor_tensor(
                                        dst, ps1[:], b1_sb[:, 1:2],
                                        _r(zero1, [[zero1.ap[0][0], 128], [0, 400]]),
                                        AL.add, AL.max,
                                    )

                    def conv2_chain(half, co, pc2, group, ci):
                        """Half of a K=256 conv2 PSUM chain (one ci block)."""
                        g0, nb = group
                        key = (half, co, g0)
                        if ci == 0:
                            ps2 = ps2p.tile([128, 504], F32, tag="ps2")
                            _ps2_open[key] = ps2
                        else:
                            ps2 = _ps2_open.pop(key)
                        pstep = ps2.ap[0][0]
                        wt = w2c[(co, ci)]
                        hp = h1[ci].ap[0][0]
                        for kk in range(81):
                            ky, kx = divmod(kk, 9)
                            rhs = AP(h1[ci].tensor,
                                     h1[ci].offset + g0 * 400 + ky * 20 + kx,
                                     [[hp, 128], [400, nb], [40, 6], [2, 6]])
                            out4 = _r(ps2, [[pstep, 128], [36, nb],
                                            [6, 6], [1, 6]])
                            nc.tensor.matmul(
                                out4,
                                wt[:, kk * 128:(kk + 1) * 128],
                                rhs,
                                start=(ci == 0 and kk == 0),
                                stop=(ci == 1 and kk == 80),
                            )
                        if ci == 1:
                            # bias+relu, pix-major into pc2 [p, pix*32+b]
                            nc.scalar.activation(
                                AP(pc2.tensor, pc2.offset + g0,
                                   [[pc2.ap[0][0], 128], [1, nb], [HB, 36]]),
                                _r(ps2, [[pstep, 128], [36, nb], [1, 36]]),
                                AF.Relu,
                                bias=pcb_sb[:, co:co + 1],
                            )

                    _ps2_open = {}

                    def conv2_group(half, co, pc2, g0, nb):
                        conv2_chain(half, co, pc2, (g0, nb), ci=0)
                        conv2_chain(half, co, pc2, (g0, nb), ci=1)

                    def store_pc2(half, co, pc2):
                        nc.sync.dma_start(
                            AP(pc_rd[:].tensor,
                               co * 128 * 36 * BC + half * HB,
                               [[36 * BC, 128], [BC, 36], [1, HB]]),
                            AP(pc2.tensor, pc2.offset,
                               [[pc2.ap[0][0], 128], [HB, 36], [1, HB]]),
                        )

                    def conv_half(half, pws, mid=None):
                        # conv1 images 0..13 (chunks 0,1)
                        for (pw, lo, n) in pws[:2]:
                            conv1_imgs(pw, lo, n)
                        pc2 = pc2p.tile([128, HB * 36], BF16, tag="pc2")
                        # g0 (images 0..13) as soon as w2c00 lands;
                        # conv1 images 14..27 interleave between its chains
                        conv2_chain(half, 0, pc2, GROUPS[0], ci=0)
                        for (pw, lo, n) in pws[2:4]:
                            conv1_imgs(pw, lo, n)
                        conv2_chain(half, 0, pc2, GROUPS[0], ci=1)
                        conv2_group(half, 0, pc2, *GROUPS[1])
                        for (pw, lo, n) in pws[4:]:
                            conv1_imgs(pw, lo, n)
                        conv2_group(half, 0, pc2, *GROUPS[2])
                        store_pc2(half, 0, pc2)
                        if mid is not None:
                            mid()
                        pc2 = pc2p.tile([128, HB * 36], BF16, tag="pc2")
                        for (g0, nb) in GROUPS:
                            conv2_group(half, 1, pc2, g0, nb)
                        return pc2

                    # ---- half 0 (patch DMAs interleaved with w2c loads) ----
                    pws0 = []
                    for (lo, n) in PATCH_CHUNKS[:2]:
                        pws0.append((patch_dma(0, lo, n), lo, n))
                    load_w2c(0, 0)
                    for (lo, n) in PATCH_CHUNKS[2:4]:
                        pws0.append((patch_dma(0, lo, n), lo, n))
                    load_w2c(0, 1)
                    pws0.append((patch_dma(0, *PATCH_CHUNKS[4]), *PATCH_CHUNKS[4]))

                    def mid0():
                        # co1 weights + routing prefetches, issued after the
                        # half-0 co0 section in program order
                        load_w2c(1, 0)
                        load_w2c(1, 1)
                        nc.sync.dma_start(w2sb[:], w2ns[:])
                        nc.sync.dma_start(eye_sb[:], eye64[:])

                    pc2_last = conv_half(0, pws0, mid=mid0)
                    pws1 = [(patch_dma(1, lo, n), lo, n)
                            for (lo, n) in PATCH_CHUNKS[:2]]
                    store_pc2(0, 1, pc2_last)
                    # ---- half 1 (first two patch chunks pre-issued) ----
                    pws1 += [(patch_dma(1, lo, n), lo, n)
                             for (lo, n) in PATCH_CHUNKS[2:]]
                    pc2_last = conv_half(1, pws1)
                    # xrT first half: co 0 rows (r-tiles 0..35) complete
                    nc.sync.dma_start(
                        xrT_a[:],
                        AP(pc_rd[:].tensor, 0,
                           [[BC, 128], [128 * BC, HT], [1, BC]]),
                    )
                    # w2nt into the conv-weight slots freed by co 0
                    w2nt_a = wbig.tile([128, NT * 128], BF16, tag="wslot",
                                       bufs=4, name="w2nt_a")
                    nc.sync.dma_start(
                        w2nt_a[:],
                        AP(w2nt[:].tensor, 0, [[NS, 128], [128, NT], [1, 128]]),
                    )
                    w2nt_b = wbig.tile([32, NT * 128], BF16, tag="wslot",
                                       bufs=4, name="w2nt_b")
                    nc.sync.dma_start(
                        w2nt_b[:],
                        AP(w2nt[:].tensor, 128 * NS, [[NS, 32], [128, NT], [1, 128]]),
                    )
                    # last (co1, half1) store + xrT second half, split in
                    # 32-partition groups so the s-chain tail streams in
                    p2l = pc2_last.ap[0][0]
                    for q in range(4):
                        nc.sync.dma_start(
                            AP(pc_rd[:].tensor,
                               (128 + 32 * q) * 36 * BC + HB,
                               [[36 * BC, 32], [BC, 36], [1, HB]]),
                            AP(pc2_last.tensor, pc2_last.offset + 32 * q * p2l,
                               [[p2l, 32], [HB, 36], [1, HB]]),
                        )
                        nc.sync.dma_start(
                            xrT_b[:, q * 9 * BC:(q + 1) * 9 * BC],
                            AP(pc_rd[:].tensor, (HT + q * 9) * 128 * BC,
                               [[BC, 128], [128 * BC, 9], [1, BC]]),
                        )

                # ---------------- routing phase ----------------
                with (
                    tc.tile_pool(name="rnd", bufs=2) as rnd,
                    tc.tile_pool(name="sps", bufs=1, space="PSUM") as sps,
                    tc.tile_pool(name="gps", bufs=3, space="PSUM") as gps,
                    tc.tile_pool(name="vps", bufs=1, space="PSUM") as vps,
                    tc.tile_pool(name="zps", bufs=1, space="PSUM") as zps,
                ):
                    def s_matmul(xst_of):
                        s_ps = sps.tile([BC, HL], F32, tag="s_ps")
                        for t in range(NT):
                            nc.tensor.matmul(
                                s_ps[:],
                                xst_of(t),
                                w2sb[:, t * HL:(t + 1) * HL],
                                start=(t == 0), stop=(t == NT - 1),
                            )
                        return s_ps

                    def xr_slice(t):
                        src = xrT_a if t < HT else xrT_b
                        tt = t % HT
                        return src[:, tt * BC:(tt + 1) * BC]

                    def squash(s_ps, out_dtype, scale):
                        """v = squash(s_ps*scale) over the class dim."""
                        sq = rnd.tile([BC, HL], F32, tag="sq")
                        nc.scalar.activation(sq[:], s_ps[:], AF.Square,
                                             scale=scale)
                        n2 = rnd.tile([BC, 16], F32, tag="n2")
                        nc.vector.tensor_reduce(
                            n2[:].rearrange("a b -> a b ()"),
                            _r(sq, [[sq.ap[0][0], BC], [1, 16], [16, 10]]),
                            AX.X, AL.add,
                        )
                        rt = rnd.tile([BC, 16], F32, tag="rt")
                        nc.scalar.sqrt(rt[:], n2[:])
                        n2p1 = rnd.tile([BC, 16], F32, tag="n2p1")
                        nc.vector.tensor_scalar_add(n2p1[:], n2[:], 1.0)
                        rcp = rnd.tile([BC, 16], F32, tag="rcp")
                        nc.vector.reciprocal(rcp[:], n2p1[:])
                        f = rnd.tile([BC, 16], F32, tag="f")
                        nc.vector.tensor_tensor(f[:], rt[:], rcp[:], AL.mult)
                        v_sb = rnd.tile([BC, HL], out_dtype, tag="v_sb")
                        nc.vector.scalar_tensor_tensor(
                            _r(v_sb, [[v_sb.ap[0][0], BC], [16, 10], [1, 16]]),
                            _r(s_ps, [[s_ps.ap[0][0], BC], [16, 10], [1, 16]]),
                            scale,
                            _r(f, [[f.ap[0][0], BC], [0, 10], [1, 16]]),
                            AL.mult, AL.mult,
                        )
                        return v_sb

                    def delta_update(v_bf, rnd_idx):
                        """b9 (+)= ROUTE_SCALE * allreduce(sum_b xrT*P)."""
                        vt_ps = vps.tile([128, BC], BF16, tag="vt_ps")
                        nc.tensor.transpose(vt_ps[:], v_bf[:, 0:128], eye_sb[:])
                        vt_a = rnd.tile([128, BC], BF16, tag="vt_a")
                        nc.scalar.copy(vt_a[:], vt_ps[:])
                        vtb_ps = vps.tile([32, BC], BF16, tag="vtb_ps")
                        nc.tensor.transpose(vtb_ps[:], v_bf[:, 128:160], eye_sb[:])
                        vt_b = rnd.tile([32, BC], BF16, tag="vt_b")
                        nc.scalar.copy(vt_b[:], vtb_ps[:])
                        delta9 = rnd.tile([128, 9], F32, tag="delta9")
                        acc2 = rnd.tile([128, 18], F32, tag="acc2")
                        # Group P tiles by j = t%9 so a fused multiply+sum per
                        # (j,half) yields delta9[:,j] partials directly.
                        for j in range(9):
                            for half, src_x in ((0, xrT_a), (1, xrT_b)):
                                pph = gps.tile([128, 4 * BC], F32, tag="pph")
                                for si in range(4):
                                    t = (half * 4 + si) * 9 + j
                                    nc.tensor.matmul(
                                        pph[:, si * BC:(si + 1) * BC],
                                        w2nt_a[:, t * 128:(t + 1) * 128],
                                        vt_a[:],
                                        start=True, stop=False,
                                    )
                                    nc.tensor.matmul(
                                        pph[:, si * BC:(si + 1) * BC],
                                        w2nt_b[:, t * 128:(t + 1) * 128],
                                        vt_b[:],
                                        start=False, stop=True,
                                    )
                                prodh = rnd.tile([128, 4 * BC], BF16, tag="prodh")
                                in1 = AP(src_x.tensor, src_x.offset + j * BC,
                                         [[src_x.ap[0][0], 128], [9 * BC, 4],
                                          [1, BC]])
                                in0 = _r(pph, [[pph.ap[0][0], 128], [BC, 4],
                                               [1, BC]])
                                out3 = _r(prodh, [[prodh.ap[0][0], 128],
                                                  [BC, 4], [1, BC]])
                                nc.vector.scalar_tensor_tensor(
                                    out3, in0, 1.0, in1, AL.mult, AL.mult,
                                    accum_out=acc2[:, j * 2 + half:
                                                   j * 2 + half + 1],
                                )
                        nc.vector.tensor_reduce(
                            delta9[:].rearrange("a b -> a b ()"),
                            _r(acc2, [[acc2.ap[0][0], 128], [2, 9], [1, 2]]),
                            AX.X, AL.add,
                        )
                        # exchange: ReduceScatter over an 8x-replicated input
                        # -> every core receives the full summed delta
                        crep = dpool.tile([NCORES, 128 * 9], F32,
                                          name=f"crep{rnd_idx}")
                        cd = dpool.tile([128 * 9], F32, name=f"cd{rnd_idx}")
                        nc.sync.dma_start(
                            AP(crep[:].tensor, 0,
                               [[9, 128], [128 * 9, NCORES], [1, 9]]),
                            _r(delta9, [[delta9.ap[0][0], 128],
                                        [0, NCORES], [1, 9]]),
                        )
                        nc.gpsimd.collective_compute(
                            "ReduceScatter", AL.add,
                            replica_groups=[list(range(NCORES))],
                            ins=[crep.opt()], outs=[cd.opt()],
                        )
                        dsum = rnd.tile([128, 9], F32, tag="dsum")
                        nc.sync.dma_start(
                            dsum[:],
                            AP(cd.tensor, cd.offset, [[9, 128], [1, 9]]),
                        )
                        if rnd_idx == 0:
                            nc.scalar.mul(b9[:], dsum[:], ROUTE_SCALE)
                        else:
                            sc = rnd.tile([128, 9], F32, tag="sc")
                            nc.scalar.mul(sc[:], dsum[:], ROUTE_SCALE)
                            nc.vector.tensor_tensor(b9[:], b9[:], sc[:], AL.add)

                    def softmax_ce9b():
                        """ce9b[p,j] = softmax(b9)[n=j*128+p], BF16 (128,9)."""
                        e9 = rnd.tile([128, 9], F32, tag="e9")
                        rs9 = rnd.tile([128, 1], F32, tag="rs9")
                        nc.scalar.activation(e9[:], b9[:], AF.Exp,
                                             accum_out=rs9[:])
                        z_ps = zps.tile([1, 1], F32, tag="z_ps")
                        nc.tensor.matmul(z_ps[:], ones128[:], rs9[:],
                                         start=True, stop=True)
                        z_sb = rnd.tile([1, 1], F32, tag="z_sb")
                        nc.scalar.copy(z_sb[:], z_ps[:])
                        zb_ps = zps.tile([128, 1], F32, tag="zb_ps")
                        nc.tensor.matmul(zb_ps[:], ones1[:], z_sb[:],
                                         start=True, stop=True)
                        rz = rnd.tile([128, 1], F32, tag="rz")
                        nc.vector.reciprocal(rz[:], zb_ps[:])
                        ce9b = rnd.tile([128, 9], BF16, tag="ce9b")
                        nc.vector.tensor_scalar_mul(ce9b[:], e9[:], rz[:])
                        return ce9b

                    def scaled_x(ce9b):
                        """xc[p,(s,j,b)] = xrT * ce9b[p,j], bf16 copies.
                        Quarter ops: DVE feeds the s-chain head while Pool
                        scales the second half concurrently."""
                        xc_a = rnd.tile([128, HT * BC], BF16, tag="xc_a")
                        xc_b = rnd.tile([128, HT * BC], BF16, tag="xc_b")
                        for eng, xch, src, q in ((nc.vector, xc_a, xrT_a, 0),
                                                 (nc.gpsimd, xc_b, xrT_b, 0),
                                                 (nc.vector, xc_a, xrT_a, 1),
                                                 (nc.gpsimd, xc_b, xrT_b, 1)):
                            off = q * 2 * 9 * BC
                            eng.tensor_tensor(
                                AP(xch.tensor, xch.offset + off,
                                   [[xch.ap[0][0], 128], [9 * BC, 2],
                                    [BC, 9], [1, BC]]),
                                AP(src.tensor, src.offset + off,
                                   [[src.ap[0][0], 128], [9 * BC, 2],
                                    [BC, 9], [1, BC]]),
                                _r(ce9b, [[ce9b.ap[0][0], 128], [0, 2],
                                          [1, 9], [0, BC]]),
                                AL.mult,
                            )
                        return xc_a, xc_b

                    # ---- round 0 (c uniform) ----
                    s_ps = s_matmul(xr_slice)
                    v_bf = squash(s_ps, BF16, 1.0 / 1152.0)
                    delta_update(v_bf, 0)
                    # ---- round 1 ----
                    ce9b = softmax_ce9b()
                    xc_a, xc_b = scaled_x(ce9b)
                    s_ps = s_matmul(
                        lambda t: (xc_a if t < HT else xc_b)
                        [:, (t % HT) * BC:(t % HT + 1) * BC])
                    v_bf = squash(s_ps, BF16, 1.0)
                    delta_update(v_bf, 1)
                    # ---- round 2 (b update dead) ----
                    ce9b = softmax_ce9b()
                    xc_a, xc_b = scaled_x(ce9b)
                    s_ps = s_matmul(
                        lambda t: (xc_a if t < HT else xc_b)
                        [:, (t % HT) * BC:(t % HT + 1) * BC])
                    v_sb = squash(s_ps, F32, 1.0)
                    nc.sync.dma_start(vout[:], v_sb[:])

    return nc


_NC_CACHE = None


def _get_nc():
    global _NC_CACHE
    if _NC_CACHE is None:
        nc = build_nc()
        split_waits(nc)
        _NC_CACHE = nc
    return _NC_CACHE


def prepare_inputs(x, conv1_w, conv1_b, pc_w, pc_b, W):
    bf = mybir.dt.np(BF16)
    x = np.asarray(x, np.float32)
    xs = np.zeros((B, 800), np.float32)
    xs[:, :784] = x.reshape(B, 784)
    # host-side patch expansion: xp[kk, b, e] = xs[b, (kk//9)*28 + kk%9 + e]
    kidx = (np.arange(9)[:, None] * 28 + np.arange(9)[None, :]).reshape(81)
    xp = np.stack([xs[:, k:k + 560] for k in kidx], 0).astype(bf)  # [81, B, 560]
    w1t = np.ascontiguousarray(
        np.asarray(conv1_w, np.float32).reshape(256, 81).T).astype(bf)
    b1 = np.ascontiguousarray(np.asarray(conv1_b, np.float32))
    pcwt = np.asarray(pc_w, np.float32).reshape(256, 256, 81).transpose(2, 1, 0)
    # pcw4[co*2+ci][p, kk*128+co_p] = pcwt[kk, ci*128+p, co*128+co_p]
    pcw4 = np.stack([
        np.ascontiguousarray(
            pcwt[:, ci * 128:(ci + 1) * 128, co * 128:(co + 1) * 128]
            .transpose(1, 0, 2).reshape(128, 81 * 128))
        for (co, ci) in [(0, 0), (0, 1), (1, 0), (1, 1)]
    ], 0).astype(bf)
    pcb = np.ascontiguousarray(np.asarray(pc_b, np.float32).reshape(256))
    w2n = np.ascontiguousarray(
        np.asarray(W, np.float32).transpose(3, 0, 1, 2).reshape(NS, HL))
    # w2ns[p, t*HL+hl] = w2n[t*128+p, hl]
    w2ns = np.ascontiguousarray(
        w2n.reshape(NT, 128, HL).transpose(1, 0, 2).reshape(128, NT * HL)
    ).astype(bf)
    w2nt = np.ascontiguousarray(w2n.T).astype(bf)
    eye64 = np.eye(BC, dtype=np.float32).astype(bf)
    in_maps = []
    for c in range(NCORES):
        in_maps.append({
            "xp": np.ascontiguousarray(xp[:, c * BC:(c + 1) * BC, :]),
            "w1t": w1t, "b1": b1, "pcw4": pcw4, "pcb": pcb, "w2ns": w2ns,
            "w2nt": w2nt, "eye64": eye64,
        })
    return in_maps


def finalize_output(results):
    v = np.concatenate([np.asarray(results[c]["vout"]) for c in range(NCORES)], 0)
    return v.reshape(B, 1, 1, 10, 16).astype(np.float32)


def kernel(x, conv1_w, conv1_b, pc_w, pc_b, W, _trace=False, _trace_kwargs=None):
    nc = _get_nc()
    in_maps = prepare_inputs(x, conv1_w, conv1_b, pc_w, pc_b, W)
    res = run_bass_kernel_spmd(
        nc, in_maps, list(range(NCORES)),
        trace=_trace, **(_trace_kwargs or {}),
    )
    out = finalize_output(res.results)
    if _trace:
        return out, res
    return out


# revision 4
# speedup vs baseline: 1.8072x; 1.8072x over previous
"""CapsNet forward kernel for Trainium2, 8-core data-parallel (v2, bf16).

Strategy: batch (512) split across 8 cores (64 each); params replicated.
Routing logits b are a batch-mean -> AllGather of per-core partial deltas
(1152 floats) per routing round (rounds 0,1; round 2's b update is dead).

v2 changes vs baseline:
  - all PE matmuls in bf16 (same streaming rate as fp32r at N>=256, 4x
    faster at the routing's small-N matmuls; half the DMA bytes).
  - conv runs in two 32-image halves so h1 (both ci blocks, bf16) fits
    in SBUF; conv2 accumulates K=256 in a single PSUM chain (no acc add).
  - conv1 bias+relu alternates Act/DVE engines; conv2 group 0 is emitted
    between conv1 chunks so the PE starts conv2 as soon as its inputs
    land instead of idling on patch DMAs.
  - patch rows pre-expanded on host (xp[81,b,560]) so patch DMAs are
    3-dim and batched; half-1 patches prefetched during half 0.
  - routing weights (w2nt) DMA'd into the conv weight pool's freed slots
    during the last conv section; xrT round-trip split in two tiles and
    overlapped with the last conv stores.
  - P*xr reduction grouped 8 r-tiles per PSUM bank, consumed by two DVE
    ops per group (no per-tile PSUM->SBUF copies).
  - per-round scaled bf16 copies of xrT (master stays unscaled -> no
    reciprocal un-scaling).
"""

import numpy as np

import concourse.bass as bass
import concourse.mybir as mybir
import concourse.tile as tile
from concourse.ap import AP
from concourse.bass_utils import run_bass_kernel_spmd

F32 = mybir.dt.float32
BF16 = mybir.dt.bfloat16
AL = mybir.AluOpType
AF = mybir.ActivationFunctionType
AX = mybir.AxisListType

NCORES = 8
B = 512
BC = B // NCORES           # 64 images per core
HB = BC // 2               # 32 images per conv half
MAX_WAITS = 1              # walrus on this path allows 1 sync wait per inst
HL = 160                   # 10 classes x 16 pose
NS = 9216                  # 1152 caps x 8
NT = NS // 128             # 72 K-tiles
HT = NT // 2               # 36 K-tiles per xrT half
GROUPS = [(0, 14), (14, 14), (28, 4)]   # conv2 image groups per half
PATCH_CHUNKS = [(0, 7), (7, 7), (14, 7), (21, 7), (28, 4)]  # conv1 DMA chunks
ROUTE_SCALE = 1.0 / (B * HL)


def _r(t, dims):
    """Raw AP on tile/ap t with explicit [step, count] dims (elements)."""
    return AP(t.tensor, t.offset, dims)


def split_waits(nc, max_waits=MAX_WAITS):
    """This walrus build rejects >max_waits sync waits per instruction; move
    excess waits onto same-engine NoOps inserted immediately before."""
    for f in nc.m.functions:
        for blk in f.blocks:
            out = []
            for ins in blk.instructions:
                si = ins.sync_info
                if si is not None and si.on_wait and len(si.on_wait) > max_waits:
                    waits = list(si.on_wait)
                    k = 0
                    while len(waits) > max_waits:
                        chunk, waits = waits[:max_waits], waits[max_waits:]
                        nop = mybir.InstNoOp(name=f"{ins.name}-ws{k}", ins=[], outs=[])
                        nop.engine = ins.engine
                        nop.sync_info = mybir.SyncInfo(on_wait=chunk, on_update=[])
                        out.append(nop)
                        k += 1
                    ins.sync_info = mybir.SyncInfo(
                        on_wait=waits, on_update=list(si.on_update or []))
                out.append(ins)
            blk.instructions = out


def build_nc():
    nc = bass.Bass(num_devices=NCORES)

    xp = nc.dram_tensor("xp", [81, BC, 560], BF16, kind="ExternalInput")
    w1t = nc.dram_tensor("w1t", [81, 256], BF16, kind="ExternalInput")
    b1 = nc.dram_tensor("b1", [256], F32, kind="ExternalInput")
    pcw4 = nc.dram_tensor("pcw4", [4, 128, 81 * 128], BF16, kind="ExternalInput")
    pcb = nc.dram_tensor("pcb", [256], F32, kind="ExternalInput")
    w2ns = nc.dram_tensor("w2ns", [128, NT * HL], BF16, kind="ExternalInput")
    w2nt = nc.dram_tensor("w2nt", [HL, NS], BF16, kind="ExternalInput")
    eye64 = nc.dram_tensor("eye64", [BC, BC], BF16, kind="ExternalInput")
    vout = nc.dram_tensor("vout", [BC, HL], F32, kind="ExternalOutput")

    pc_rd = nc.dram_tensor("pc_rd", [NS, BC], BF16)    # [r, b]

    with tile.TileContext(nc) as tc:
        with (
            tc.tile_pool(name="pers", bufs=1) as pers,
            tc.tile_pool(name="dram", bufs=1, space="DRAM") as dpool,
        ):
            w1t_sb = pers.tile([81, 256], BF16)
            nc.sync.dma_start(w1t_sb[:], w1t[:])
            b1_sb = pers.tile([128, 2], F32)
            pcb_sb = pers.tile([128, 2], F32)
            eye_sb = pers.tile([BC, BC], BF16)
            zero1 = pers.tile([128, 1], F32)
            nc.gpsimd.memset(zero1[:], 0.0)
            ones128 = pers.tile([128, 1], F32)
            nc.gpsimd.memset(ones128[:], 1.0)
            ones1 = pers.tile([1, 128], F32)
            nc.gpsimd.memset(ones1[:], 1.0)
            b9 = pers.tile([128, 9], F32)
            # big persistent routing tensors (DMAs issued later, mid-conv)
            w2sb = pers.tile([128, NT * HL], BF16)
            xrT_a = pers.tile([128, HT * BC], BF16)
            xrT_b = pers.tile([128, HT * BC], BF16)

            with tc.tile_pool(name="wbig", bufs=1) as wbig:
                # ---------------- conv phase ----------------
                with (
                    tc.tile_pool(name="convsb", bufs=1) as csb,
                    tc.tile_pool(name="pwp", bufs=2) as pwp,
                    tc.tile_pool(name="pc2p", bufs=2) as pc2p,
                    tc.tile_pool(name="ps1p", bufs=4, space="PSUM") as ps1p,
                    tc.tile_pool(name="ps2p", bufs=2, space="PSUM") as ps2p,
                ):
                    h1_0 = csb.tile([128, HB * 400], BF16)
                    h1_1 = csb.tile([128, HB * 400], BF16)
                    h1 = [h1_0, h1_1]
                    w2c = {}
                    for (co, ci) in [(0, 0), (0, 1), (1, 0), (1, 1)]:
                        t = wbig.tile([128, 81 * 128], BF16, tag="wslot",
                                      bufs=4, name=f"w2c_{co}{ci}")
                        w2c[(co, ci)] = t

                    def load_w2c(co, ci):
                        nc.sync.dma_start(
                            w2c[(co, ci)][:],
                            AP(pcw4[:].tensor, (co * 2 + ci) * 128 * 81 * 128,
                               [[81 * 128, 128], [1, 81 * 128]]),
                        )

                    def patch_dma(half, lo, n):
                        """DMA patch rows for images half*32+lo .. +n."""
                        pw = pwp.tile([81, 8 * 560], BF16, tag="pw")
                        nc.sync.dma_start(
                            _r(pw, [[pw.ap[0][0], 81], [1, n * 560]]),
                            AP(xp[:].tensor, (half * HB + lo) * 560,
                               [[BC * 560, 81], [560, n], [1, 560]]),
                        )
                        return pw

                    def conv1_imgs(pw, lo, n):
                        """conv1 matmuls/acts for the n images in patch pw."""
                        pwstep = pw.ap[0][0]
                        for j in range(n):
                            for ci in range(2):
                                ps1 = ps1p.tile([128, 400], F32, tag="ps1")
                                rhs = AP(pw.tensor, pw.offset + j * 560,
                                         [[pwstep, 81], [28, 20], [1, 20]])
                                out4 = _r(ps1, [[ps1.ap[0][0], 128],
                                                [20, 20], [1, 20]])
                                nc.tensor.matmul(
                                    out4,
                                    w1t_sb[:, ci * 128:(ci + 1) * 128],
                                    rhs,
                                    start=True, stop=True,
                                )
                                il = lo + j
                                dst = h1[ci][:, il * 400:(il + 1) * 400]
                                if ci == 0:
                                    nc.scalar.activation(
                                        dst, ps1[:], AF.Relu,
                                        bias=b1_sb[:, 0:1],
                                    )
                                else:
                                    nc.vector.scalar_tensor_tensor(
                                        dst, ps1[:], b1_sb[:, 1:2],
                                        _r(zero1, [[zero1.ap[0][0], 128], [0, 400]]),
                                        AL.add, AL.max,
                                    )

                    def conv2_chain(half, co, pc2, group, ci):
                        """Half of a K=256 conv2 PSUM chain (one ci block)."""
                        g0, nb = group
                        key = (half, co, g0)
                        if ci == 0:
                            ps2 = ps2p.tile([128, 504], F32, tag="ps2")
                            _ps2_open[key] = ps2
                        else:
                            ps2 = _ps2_open.pop(key)
                        pstep = ps2.ap[0][0]
                        wt = w2c[(co, ci)]
                        hp = h1[ci].ap[0][0]
                        for kk in range(81):
                            ky, kx = divmod(kk, 9)
                            rhs = AP(h1[ci].tensor,
                                     h1[ci].offset + g0 * 400 + ky * 20 + kx,
                                     [[hp, 128], [400, nb], [40, 6], [2, 6]])
                            out4 = _r(ps2, [[pstep, 128], [36, nb],
                                            [6, 6], [1, 6]])
                            nc.tensor.matmul(
                                out4,
                                wt[:, kk * 128:(kk + 1) * 128],
                                rhs,
                                start=(ci == 0 and kk == 0),
                                stop=(ci == 1 and kk == 80),
                            )
                        if ci == 1:
                            # bias+relu, pix-major into pc2 [p, pix*32+b]
                            nc.scalar.activation(
                                AP(pc2.tensor, pc2.offset + g0,
                                   [[pc2.ap[0][0], 128], [1, nb], [HB, 36]]),
                                _r(ps2, [[pstep, 128], [36, nb], [1, 36]]),
                                AF.Relu,
                                bias=pcb_sb[:, co:co + 1],
                            )

                    _ps2_open = {}

                    def conv2_group(half, co, pc2, g0, nb):
                        conv2_chain(half, co, pc2, (g0, nb), ci=0)
                        conv2_chain(half, co, pc2, (g0, nb), ci=1)

                    def store_pc2(half, co, pc2):
                        nc.sync.dma_start(
                            AP(pc_rd[:].tensor,
                               co * 128 * 36 * BC + half * HB,
                               [[36 * BC, 128], [BC, 36], [1, HB]]),
                            AP(pc2.tensor, pc2.offset,
                               [[pc2.ap[0][0], 128], [HB, 36], [1, HB]]),
                        )

                    def conv_half(half, pws, mid=None):
                        # conv1 images 0..13 (chunks 0,1)
                        for (pw, lo, n) in pws[:2]:
                            conv1_imgs(pw, lo, n)
                        pc2 = pc2p.tile([128, HB * 36], BF16, tag="pc2")
                        # g0 (images 0..13) as soon as w2c00 lands;
                        # conv1 images 14..27 interleave between its chains
                        conv2_chain(half, 0, pc2, GROUPS[0], ci=0)
                        for (pw, lo, n) in pws[2:4]:
                            conv1_imgs(pw, lo, n)
                        conv2_chain(half, 0, pc2, GROUPS[0], ci=1)
                        conv2_group(half, 0, pc2, *GROUPS[1])
                        for (pw, lo, n) in pws[4:]:
                            conv1_imgs(pw, lo, n)
                        conv2_group(half, 0, pc2, *GROUPS[2])
                        store_pc2(half, 0, pc2)
                        if mid is not None:
                            mid()
                        pc2 = pc2p.tile([128, HB * 36], BF16, tag="pc2")
                        for (g0, nb) in GROUPS:
                            conv2_group(half, 1, pc2, g0, nb)
                        return pc2

                    # ---- half 0 (patch DMAs interleaved with w2c loads) ----
                    pws0 = []
                    for (lo, n) in PATCH_CHUNKS[:2]:
                        pws0.append((patch_dma(0, lo, n), lo, n))
                    nc.sync.dma_start(b1_sb[:], _r(b1[:], [[1, 128], [128, 2]]))
                    nc.sync.dma_start(pcb_sb[:], _r(pcb[:], [[1, 128], [128, 2]]))
                    load_w2c(0, 0)
                    for (lo, n) in PATCH_CHUNKS[2:4]:
                        pws0.append((patch_dma(0, lo, n), lo, n))
                    load_w2c(0, 1)
                    pws0.append((patch_dma(0, *PATCH_CHUNKS[4]), *PATCH_CHUNKS[4]))

                    def mid0():
                        # co1 weights + routing prefetches, issued after the
                        # half-0 co0 section in program order
                        load_w2c(1, 0)
                        load_w2c(1, 1)
                        nc.sync.dma_start(w2sb[:], w2ns[:])
                        nc.sync.dma_start(eye_sb[:], eye64[:])

                    pc2_last = conv_half(0, pws0, mid=mid0)
                    pws1 = [(patch_dma(1, lo, n), lo, n)
                            for (lo, n) in PATCH_CHUNKS[:2]]
                    store_pc2(0, 1, pc2_last)
                    # ---- half 1 (first two patch chunks pre-issued) ----
                    pws1 += [(patch_dma(1, lo, n), lo, n)
                             for (lo, n) in PATCH_CHUNKS[2:]]
                    pc2_last = conv_half(1, pws1)
                    # xrT first half: co 0 rows (r-tiles 0..35) complete
                    nc.sync.dma_start(
                        xrT_a[:],
                        AP(pc_rd[:].tensor, 0,
                           [[BC, 128], [128 * BC, HT], [1, BC]]),
                    )
                    # w2nt into the conv-weight slots freed by co 0
                    w2nt_a = wbig.tile([128, NT * 128], BF16, tag="wslot",
                                       bufs=4, name="w2nt_a")
                    nc.sync.dma_start(
                        w2nt_a[:],
                        AP(w2nt[:].tensor, 0, [[NS, 128], [128, NT], [1, 128]]),
                    )
                    w2nt_b = wbig.tile([32, NT * 128], BF16, tag="wslot",
                                       bufs=4, name="w2nt_b")
                    nc.sync.dma_start(
                        w2nt_b[:],
                        AP(w2nt[:].tensor, 128 * NS, [[NS, 32], [128, NT], [1, 128]]),
                    )
                    # last (co1, half1) store + xrT second half, split in
                    # 32-partition groups so the s-chain tail streams in
                    p2l = pc2_last.ap[0][0]
                    for q in range(4):
                        nc.sync.dma_start(
                            AP(pc_rd[:].tensor,
                               (128 + 32 * q) * 36 * BC + HB,
                               [[36 * BC, 32], [BC, 36], [1, HB]]),
                            AP(pc2_last.tensor, pc2_last.offset + 32 * q * p2l,
                               [[p2l, 32], [HB, 36], [1, HB]]),
                        )
                        nc.sync.dma_start(
                            xrT_b[:, q * 9 * BC:(q + 1) * 9 * BC],
                            AP(pc_rd[:].tensor, (HT + q * 9) * 128 * BC,
                               [[BC, 128], [128 * BC, 9], [1, BC]]),
                        )

                # ---------------- routing phase ----------------
                with (
                    tc.tile_pool(name="rnd", bufs=2) as rnd,
                    tc.tile_pool(name="sps", bufs=1, space="PSUM") as sps,
                    tc.tile_pool(name="gps", bufs=3, space="PSUM") as gps,
                    tc.tile_pool(name="vps", bufs=1, space="PSUM") as vps,
                    tc.tile_pool(name="zps", bufs=1, space="PSUM") as zps,
                ):
                    def s_matmul(xst_of):
                        s_ps = sps.tile([BC, HL], F32, tag="s_ps")
                        for t in range(NT):
                            nc.tensor.matmul(
                                s_ps[:],
                                xst_of(t),
                                w2sb[:, t * HL:(t + 1) * HL],
                                start=(t == 0), stop=(t == NT - 1),
                            )
                        return s_ps

                    def xr_slice(t):
                        src = xrT_a if t < HT else xrT_b
                        tt = t % HT
                        return src[:, tt * BC:(tt + 1) * BC]

                    def squash(s_ps, out_dtype, scale):
                        """v = squash(s_ps*scale) over the class dim."""
                        sq = rnd.tile([BC, HL], F32, tag="sq")
                        nc.scalar.activation(sq[:], s_ps[:], AF.Square,
                                             scale=scale)
                        n2 = rnd.tile([BC, 16], F32, tag="n2")
                        nc.vector.tensor_reduce(
                            n2[:].rearrange("a b -> a b ()"),
                            _r(sq, [[sq.ap[0][0], BC], [1, 16], [16, 10]]),
                            AX.X, AL.add,
                        )
                        rt = rnd.tile([BC, 16], F32, tag="rt")
                        nc.scalar.sqrt(rt[:], n2[:])
                        n2p1 = rnd.tile([BC, 16], F32, tag="n2p1")
                        nc.vector.tensor_scalar_add(n2p1[:], n2[:], 1.0)
                        rcp = rnd.tile([BC, 16], F32, tag="rcp")
                        nc.vector.reciprocal(rcp[:], n2p1[:])
                        f = rnd.tile([BC, 16], F32, tag="f")
                        nc.vector.tensor_tensor(f[:], rt[:], rcp[:], AL.mult)
                        v_sb = rnd.tile([BC, HL], out_dtype, tag="v_sb")
                        nc.vector.scalar_tensor_tensor(
                            _r(v_sb, [[v_sb.ap[0][0], BC], [16, 10], [1, 16]]),
                            _r(s_ps, [[s_ps.ap[0][0], BC], [16, 10], [1, 16]]),
                            scale,
                            _r(f, [[f.ap[0][0], BC], [0, 10], [1, 16]]),
                            AL.mult, AL.mult,
                        )
                        return v_sb

                    def delta_update(v_bf, rnd_idx):
                        """b9 (+)= ROUTE_SCALE * allreduce(sum_b xrT*P)."""
                        vt_ps = vps.tile([128, BC], BF16, tag="vt_ps")
                        nc.tensor.transpose(vt_ps[:], v_bf[:, 0:128], eye_sb[:])
                        vt_a = rnd.tile([128, BC], BF16, tag="vt_a")
                        nc.scalar.copy(vt_a[:], vt_ps[:])
                        vtb_ps = vps.tile([32, BC], BF16, tag="vtb_ps")
                        nc.tensor.transpose(vtb_ps[:], v_bf[:, 128:160], eye_sb[:])
                        vt_b = rnd.tile([32, BC], BF16, tag="vt_b")
                        nc.scalar.copy(vt_b[:], vtb_ps[:])
                        delta9 = rnd.tile([128, 9], F32, tag="delta9")
                        acc2 = rnd.tile([128, 18], F32, tag="acc2")
                        # Group P tiles by j = t%9 so a fused multiply+sum per
                        # (j,half) yields delta9[:,j] partials directly.
                        for j in range(9):
                            for half, src_x in ((0, xrT_a), (1, xrT_b)):
                                pph = gps.tile([128, 4 * BC], F32, tag="pph")
                                for si in range(4):
                                    t = (half * 4 + si) * 9 + j
                                    nc.tensor.matmul(
                                        pph[:, si * BC:(si + 1) * BC],
                                        w2nt_a[:, t * 128:(t + 1) * 128],
                                        vt_a[:],
                                        start=True, stop=False,
                                    )
                                    nc.tensor.matmul(
                                        pph[:, si * BC:(si + 1) * BC],
                                        w2nt_b[:, t * 128:(t + 1) * 128],
                                        vt_b[:],
                                        start=False, stop=True,
                                    )
                                prodh = rnd.tile([128, 4 * BC], BF16, tag="prodh")
                                in1 = AP(src_x.tensor, src_x.offset + j * BC,
                                         [[src_x.ap[0][0], 128], [9 * BC, 4],
                                          [1, BC]])
                                in0 = _r(pph, [[pph.ap[0][0], 128], [BC, 4],
                                               [1, BC]])
                                out3 = _r(prodh, [[prodh.ap[0][0], 128],
                                                  [BC, 4], [1, BC]])
                                nc.vector.scalar_tensor_tensor(
                                    out3, in0, 1.0, in1, AL.mult, AL.mult,
                                    accum_out=acc2[:, j * 2 + half:
                                                   j * 2 + half + 1],
                                )
                        nc.vector.tensor_reduce(
                            delta9[:].rearrange("a b -> a b ()"),
                            _r(acc2, [[acc2.ap[0][0], 128], [2, 9], [1, 2]]),
                            AX.X, AL.add,
                        )
                        # exchange: ReduceScatter over an 8x-replicated input
                        # -> every core receives the full summed delta
                        crep = dpool.tile([NCORES, 128 * 9], F32,
                                          name=f"crep{rnd_idx}")
                        cd = dpool.tile([128 * 9], F32, name=f"cd{rnd_idx}")
                        nc.sync.dma_start(
                            AP(crep[:].tensor, 0,
                               [[9, 128], [128 * 9, NCORES], [1, 9]]),
                            _r(delta9, [[delta9.ap[0][0], 128],
                                        [0, NCORES], [1, 9]]),
                        )
                        nc.gpsimd.collective_compute(
                            "ReduceScatter", AL.add,
                            replica_groups=[list(range(NCORES))],
                            ins=[crep.opt()], outs=[cd.opt()],
                        )
                        dsum = rnd.tile([128, 9], F32, tag="dsum")
                        nc.sync.dma_start(
                            dsum[:],
                            AP(cd.tensor, cd.offset, [[9, 128], [1, 9]]),
                        )
                        if rnd_idx == 0:
                            nc.scalar.mul(b9[:], dsum[:], ROUTE_SCALE)
                        else:
                            nc.vector.scalar_tensor_tensor(
                                b9[:], dsum[:], ROUTE_SCALE, b9[:],
                                AL.mult, AL.add)

                    def softmax_ce9b():
                        """ce9b[p,j] = softmax(b9)[n=j*128+p], BF16 (128,9)."""
                        e9 = rnd.tile([128, 9], F32, tag="e9")
                        rs9 = rnd.tile([128, 1], F32, tag="rs9")
                        nc.scalar.activation(e9[:], b9[:], AF.Exp,
                                             accum_out=rs9[:])
                        z_ps = zps.tile([1, 1], F32, tag="z_ps")
                        nc.tensor.matmul(z_ps[:], ones128[:], rs9[:],
                                         start=True, stop=True)
                        z_sb = rnd.tile([1, 1], F32, tag="z_sb")
                        nc.scalar.copy(z_sb[:], z_ps[:])
                        zb_ps = zps.tile([128, 1], F32, tag="zb_ps")
                        nc.tensor.matmul(zb_ps[:], ones1[:], z_sb[:],
                                         start=True, stop=True)
                        rz = rnd.tile([128, 1], F32, tag="rz")
                        nc.vector.reciprocal(rz[:], zb_ps[:])
                        ce9b = rnd.tile([128, 9], BF16, tag="ce9b")
                        nc.vector.tensor_scalar_mul(ce9b[:], e9[:], rz[:])
                        return ce9b

                    def scaled_x(ce9b):
                        """xc[p,(s,j,b)] = xrT * ce9b[p,j], bf16 copies.
                        Quarter ops: DVE feeds the s-chain head while Pool
                        scales the second half concurrently."""
                        xc_a = rnd.tile([128, HT * BC], BF16, tag="xc_a")
                        xc_b = rnd.tile([128, HT * BC], BF16, tag="xc_b")
                        for eng, xch, src, q in ((nc.vector, xc_a, xrT_a, 0),
                                                 (nc.gpsimd, xc_b, xrT_b, 0),
                                                 (nc.vector, xc_a, xrT_a, 1),
                                                 (nc.gpsimd, xc_b, xrT_b, 1)):
                            off = q * 2 * 9 * BC
                            eng.tensor_tensor(
                                AP(xch.tensor, xch.offset + off,
                                   [[xch.ap[0][0], 128], [9 * BC, 2],
                                    [BC, 9], [1, BC]]),
                                AP(src.tensor, src.offset + off,
                                   [[src.ap[0][0], 128], [9 * BC, 2],
                                    [BC, 9], [1, BC]]),
                                _r(ce9b, [[ce9b.ap[0][0], 128], [0, 2],
                                          [1, 9], [0, BC]]),
                                AL.mult,
                            )
                        return xc_a, xc_b

                    # ---- round 0 (c uniform) ----
                    s_ps = s_matmul(xr_slice)
                    v_bf = squash(s_ps, BF16, 1.0 / 1152.0)
                    delta_update(v_bf, 0)
                    # ---- round 1 ----
                    ce9b = softmax_ce9b()
                    xc_a, xc_b = scaled_x(ce9b)
                    s_ps = s_matmul(
                        lambda t: (xc_a if t < HT else xc_b)
                        [:, (t % HT) * BC:(t % HT + 1) * BC])
                    v_bf = squash(s_ps, BF16, 1.0)
                    delta_update(v_bf, 1)
                    # ---- round 2 (b update dead) ----
                    ce9b = softmax_ce9b()
                    xc_a, xc_b = scaled_x(ce9b)
                    s_ps = s_matmul(
                        lambda t: (xc_a if t < HT else xc_b)
                        [:, (t % HT) * BC:(t % HT + 1) * BC])
                    v_sb = squash(s_ps, F32, 1.0)
                    nc.sync.dma_start(vout[:], v_sb[:])

    return nc


_NC_CACHE = None


def _get_nc():
    global _NC_CACHE
    if _NC_CACHE is None:
        nc = build_nc()
        split_waits(nc)
        _NC_CACHE = nc
    return _NC_CACHE


def prepare_inputs(x, conv1_w, conv1_b, pc_w, pc_b, W):
    bf = mybir.dt.np(BF16)
    x = np.asarray(x, np.float32)
    xs = np.zeros((B, 800), np.float32)
    xs[:, :784] = x.reshape(B, 784)
    # host-side patch expansion: xp[kk, b, e] = xs[b, (kk//9)*28 + kk%9 + e]
    kidx = (np.arange(9)[:, None] * 28 + np.arange(9)[None, :]).reshape(81)
    xp = np.stack([xs[:, k:k + 560] for k in kidx], 0).astype(bf)  # [81, B, 560]
    w1t = np.ascontiguousarray(
        np.asarray(conv1_w, np.float32).reshape(256, 81).T).astype(bf)
    b1 = np.ascontiguousarray(np.asarray(conv1_b, np.float32))
    pcwt = np.asarray(pc_w, np.float32).reshape(256, 256, 81).transpose(2, 1, 0)
    # pcw4[co*2+ci][p, kk*128+co_p] = pcwt[kk, ci*128+p, co*128+co_p]
    pcw4 = np.stack([
        np.ascontiguousarray(
            pcwt[:, ci * 128:(ci + 1) * 128, co * 128:(co + 1) * 128]
            .transpose(1, 0, 2).reshape(128, 81 * 128))
        for (co, ci) in [(0, 0), (0, 1), (1, 0), (1, 1)]
    ], 0).astype(bf)
    pcb = np.ascontiguousarray(np.asarray(pc_b, np.float32).reshape(256))
    w2n = np.ascontiguousarray(
        np.asarray(W, np.float32).transpose(3, 0, 1, 2).reshape(NS, HL))
    # w2ns[p, t*HL+hl] = w2n[t*128+p, hl]
    w2ns = np.ascontiguousarray(
        w2n.reshape(NT, 128, HL).transpose(1, 0, 2).reshape(128, NT * HL)
    ).astype(bf)
    w2nt = np.ascontiguousarray(w2n.T).astype(bf)
    eye64 = np.eye(BC, dtype=np.float32).astype(bf)
    in_maps = []
    for c in range(NCORES):
        in_maps.append({
            "xp": np.ascontiguousarray(xp[:, c * BC:(c + 1) * BC, :]),
            "w1t": w1t, "b1": b1, "pcw4": pcw4, "pcb": pcb, "w2ns": w2ns,
            "w2nt": w2nt, "eye64": eye64,
        })
    return in_maps


def finalize_output(results):
    v = np.concatenate([np.asarray(results[c]["vout"]) for c in range(NCORES)], 0)
    return v.reshape(B, 1, 1, 10, 16).astype(np.float32)


def kernel(x, conv1_w, conv1_b, pc_w, pc_b, W, _trace=False, _trace_kwargs=None):
    nc = _get_nc()
    in_maps = prepare_inputs(x, conv1_w, conv1_b, pc_w, pc_b, W)
    res = run_bass_kernel_spmd(
        nc, in_maps, list(range(NCORES)),
        trace=_trace, **(_trace_kwargs or {}),
    )
    out = finalize_output(res.results)
    if _trace:
        return out, res
    return out
